# revision 9
# baseline (speedup 1.0000x reference)
"""Trainium2 Bass kernel for nn_LocalGlobalPointAttentionBlock (N=4096, E=64K).

8-core SPMD, nodes sharded 512/core.
- Edge stage: alpha's a_dst term cancels in the segment softmax; delta folds
  into per-node tables A'=-(a_src+posW), V'=val-posW, P=posW+b_pos so each
  edge needs ONE 512-ch gather + exp + mul + one-hot-matmul scatter:
  x_pt[n] = (sum ea*V'[src])/den_eps + P[n]*(den/den_eps), ea=exp(A'[src]).
- Dense chain channel-major (weights are natural lhsT, biases are ACT
  per-partition scalars, LN stats via all-ones matmul, rstd=exp(-0.5 ln)).
- Global SDPA sequence-parallel: kT/V_aug AllGathered; V_aug carries a ones
  column per head so the softmax denominator falls out of the oT matmul.
"""
import contextlib

import numpy as np

import concourse.bass as bass
import concourse.mybir as mybir
import concourse.tile as tile
from concourse import bass_utils

F32 = mybir.dt.float32
BF = mybir.dt.bfloat16
I32 = mybir.dt.int32
AF = mybir.ActivationFunctionType
OP = mybir.AluOpType

N, E, IN_C, C = 4096, 65536, 64, 256
H, PE = 4, 32
NC, NSH, NBLK = 8, 512, 4
P = 128


def _split_waits(nc):
    """walrus here rejects >1 sync wait per instruction; hoist extras onto
    NoOps inserted before the instruction on the same engine."""
    n = [0]
    for f in nc.m.functions:
        for bb in f.blocks:
            new_list = []
            for ins in bb.instructions:
                si = ins.sync_info
                if si is not None and si.on_wait and len(si.on_wait) > 1:
                    waits = list(si.on_wait)
                    extra, keep = waits[:-1], waits[-1:]
                    for w in extra:
                        n[0] += 1
                        new_list.append(mybir.InstNoOp(
                            name=f"I-waitfix-{n[0]}", engine=ins.engine,
                            ins=[], outs=[],
                            sync_info=mybir.SyncInfo(on_wait=[w], on_update=[]),
                            text_hint="waitfix", bass_nofuse=True))
                    si.on_wait = keep
                new_list.append(ins)
            bb.instructions = new_list
    return n[0]


def _pack_host(inputs):
    ei = np.asarray(inputs["edge_index"])
    src, dst = ei[0], ei[1]
    order = np.argsort(dst, kind="stable")
    src_s, dst_s = src[order], dst[order]
    blk = dst_s // P
    counts = np.bincount(blk, minlength=32)
    CAP = int(-(-counts.max() // P) * P)
    TE = CAP // P
    NTE = NBLK * TE
    src_idx = np.zeros((NC, NTE * P), np.int32)
    dstl = np.full((NC, NTE * P), P, np.int32)
    off = np.concatenate([[0], np.cumsum(counts)])
    for b in range(32):
        ci, lb = b // NBLK, b % NBLK
        e0, e1 = off[b], off[b + 1]
        base = lb * CAP
        src_idx[ci, base:base + (e1 - e0)] = src_s[e0:e1]
        dstl[ci, base:base + (e1 - e0)] = dst_s[e0:e1] - b * P
    W_src = np.concatenate([inputs["pt_W_src"][h] for h in range(H)], 1)
    W_lin = np.concatenate([inputs["pt_W_lin"][h] for h in range(H)], 1)
    W_pos = np.concatenate([inputs["pt_W_pos"][h] for h in range(H)], 1)
    b_pos = np.asarray(inputs["pt_b_pos"]).reshape(-1)
    ct = lambda a: np.ascontiguousarray(np.asarray(a), np.float32)
    shared = {
        "xT": ct(np.asarray(inputs["x_feat"]).T),
        "posT": ct(np.asarray(inputs["pos"]).T),
        "W_AV": ct(np.concatenate([-W_src, W_lin], 1)),
        "W_P3n": ct(np.concatenate([-W_pos, -W_pos], 1)),
        "W_Ppos": ct(W_pos),
        "b_pos_rep": ct(np.tile(b_pos[None, :], (P, 1))),
        "iota_f": ct(np.tile(np.arange(P)[None, :], (P, 1))),
        "ones256": np.full((P, P), 1.0 / C, np.float32),
        "id128": np.eye(P, dtype=np.float32),
        "id64x2": ct(np.concatenate([np.eye(64)] * 2, 0)),
    }
    for nm in ["proj_W1", "proj_W2", "lm_W1", "lm_W2", "q_W", "k_W", "v_W",
               "o_W", "gm_W1", "gm_W2"]:
        shared[nm] = ct(inputs[nm])
    shared["comb_Wa"] = ct(np.asarray(inputs["comb_W"])[:C])
    shared["comb_Wb"] = ct(np.asarray(inputs["comb_W"])[C:])
    for nm in ["proj_b1", "proj_b2", "lm_b1", "lm_b2", "q_b", "k_b", "v_b",
               "o_b", "gm_b1", "gm_b2", "comb_b", "ln_l_g", "ln_l_b",
               "n1_g", "n1_b", "n2_g", "n2_b"]:
        shared[nm] = ct(np.asarray(inputs[nm]).reshape(-1))
    shared["pe_W1"] = ct(inputs["pe_W1"])
    shared["pe_W2"] = ct(inputs["pe_W2"])
    shared["pe_b1"] = ct(np.asarray(inputs["pe_b1"]).reshape(-1, 1))
    shared["pe_b2"] = ct(np.asarray(inputs["pe_b2"]).reshape(-1, 1))
    pos_np = np.asarray(inputs["pos"])
    per_core = [{
        "src_idx": src_idx[ci], "dstl": dstl[ci],
        "pos_locT": ct(pos_np[ci * NSH:(ci + 1) * NSH].T),
    } for ci in range(NC)]
    return shared, per_core, TE, NTE


def build(TE, NTE):
    nc = bass.Bass("TRN2", target_bir_lowering=False, debug=False,
                   num_devices=NC)
    di = {}

    def inp(name, shape, dt=F32):
        di[name] = nc.dram_tensor(name, list(shape), dt, kind="ExternalInput")

    inp("xT", [IN_C, N]); inp("posT", [3, N])
    inp("W_AV", [IN_C, 2 * C]); inp("W_P3n", [3, 2 * C]); inp("W_Ppos", [3, C])
    inp("b_pos_rep", [P, C]); inp("iota_f", [P, P]); inp("ones256", [P, P])
    inp("id128", [P, P]); inp("id64x2", [P, 64])
    for nm in ["proj_W1", "proj_W2", "lm_W1", "lm_W2", "q_W", "k_W", "v_W",
               "o_W", "gm_W1", "gm_W2", "comb_Wa"]:
        inp(nm, [C, C])
    inp("comb_Wb", [PE, C])
    for nm in ["proj_b1", "proj_b2", "lm_b1", "lm_b2", "q_b", "k_b", "v_b",
               "o_b", "gm_b1", "gm_b2", "comb_b", "ln_l_g", "ln_l_b",
               "n1_g", "n1_b", "n2_g", "n2_b"]:
        inp(nm, [C])
    inp("pe_W1", [3, PE]); inp("pe_W2", [PE, PE])
    inp("pe_b1", [PE, 1]); inp("pe_b2", [PE, 1])
    inp("src_idx", [NTE * P], I32); inp("dstl", [NTE * P], I32)
    inp("pos_locT", [3, NSH])
    xg_out = nc.dram_tensor("xg_out", [NSH, C], F32, kind="ExternalOutput")
    t_src = nc.dram_tensor("t_src", [N, 2 * C], BF)
    kt_sh = nc.dram_tensor("kt_sh", [2, P, NSH], BF)
    vaug_sh = nc.dram_tensor("vaug_sh", [NSH, 272], BF)
    kt_full = nc.dram_tensor("kt_full", [NC, 2, P, NSH], BF,
                             addr_space="Shared")
    vaug_full = nc.dram_tensor("vaug_full", [NC, NSH, 272], BF,
                               addr_space="Shared")

    with tile.TileContext(nc) as tc, contextlib.ExitStack() as ctx:
        con = ctx.enter_context(tc.tile_pool(name="con", bufs=1))
        chain = ctx.enter_context(tc.tile_pool(name="chain", bufs=1))
        gat = ctx.enter_context(tc.tile_pool(name="gat", bufs=4))
        scr = ctx.enter_context(tc.tile_pool(name="scr", bufs=1))
        big = ctx.enter_context(tc.tile_pool(name="big", bufs=1))
        kv = ctx.enter_context(tc.tile_pool(name="kv", bufs=4))
        pp = ctx.enter_context(tc.tile_pool(name="pp", bufs=2, space="PSUM"))
        ppt = ctx.enter_context(tc.tile_pool(name="ppt", bufs=2, space="PSUM"))
        ppo = ctx.enter_context(tc.tile_pool(name="ppo", bufs=4, space="PSUM"))

        def load(name, shape, dt=F32, rearr=None):
            t = con.tile(list(shape), dt, tag=name, name=name + "_sb")
            srcap = di[name].ap()
            if rearr:
                srcap = srcap.rearrange(rearr, p=P)
            if dt == BF:
                nc.gpsimd.dma_start(out=t[:], in_=srcap)
            else:
                nc.sync.dma_start(out=t[:], in_=srcap)
            return t

        XT = load("xT", [IN_C, N], BF)
        POST = load("posT", [3, N], BF)
        W_AV = load("W_AV", [IN_C, 2 * C], BF)
        W_P3n = load("W_P3n", [3, 2 * C], BF)
        W_Ppos = load("W_Ppos", [3, C], BF)
        b_pos_rep = load("b_pos_rep", [P, C])
        iota_f = load("iota_f", [P, P])
        ones256 = load("ones256", [P, P], BF)
        id128 = load("id128", [P, P])
        id128b = con.tile([P, P], BF, tag="id128b", name="id128b_sb")
        nc.gpsimd.dma_start(out=id128b[:], in_=di["id128"].ap())
        id64 = load("id64x2", [P, 64], BF)
        WS = {nm: load(nm, [P, 2, C], BF, rearr="(ic p) o -> p ic o")
              for nm in ["proj_W1", "proj_W2", "lm_W1", "lm_W2", "q_W",
                         "k_W", "v_W", "o_W", "gm_W1", "gm_W2", "comb_Wa"]}
        comb_Wb = load("comb_Wb", [PE, C], BF)
        BS = {nm: load(nm, [P, 2], rearr="(c p) -> p c")
              for nm in ["proj_b1", "proj_b2", "lm_b1", "lm_b2", "q_b",
                         "k_b", "v_b", "o_b", "gm_b1", "gm_b2", "comb_b",
                         "ln_l_g", "ln_l_b", "n1_g", "n1_b", "n2_g", "n2_b"]}
        pe_W1 = load("pe_W1", [3, PE], BF); pe_W2 = load("pe_W2", [PE, PE], BF)
        pe_b1 = load("pe_b1", [PE, 1]); pe_b2 = load("pe_b2", [PE, 1])
        pos_locT = load("pos_locT", [3, NSH], BF)
        srcix = load("src_idx", [P, NTE], I32, rearr="(t p) -> p t")
        dstl_i = load("dstl", [P, NTE], I32, rearr="(t p) -> p t")
        dstl_f = con.tile([P, NTE], F32, tag="dstlf", name="dstlf")
        nc.vector.tensor_copy(out=dstl_f[:], in_=dstl_i[:])
        eps_sb = con.tile([P, 1], F32, tag="eps", name="epssb")
        nc.vector.memset(eps_sb[:], 1e-5)
        eps16_sb = con.tile([P, 1], F32, tag="eps16", name="eps16sb")
        nc.vector.memset(eps16_sb[:], 1e-16)

        # ---- S2: per-node tables [A'|V'] -> t_src ----
        for nt in range(32):
            ps = pp.tile([P, 2 * C], F32, tag="ps", name="s2ps")
            sl = slice(nt * P, (nt + 1) * P)
            nc.tensor.matmul(out=ps[:], lhsT=(XT[:, sl]), rhs=(W_AV[:]),
                             start=True, stop=False, skip_group_check=True)
            nc.tensor.matmul(out=ps[:], lhsT=(POST[:, sl]), rhs=(W_P3n[:]),
                             start=False, stop=True, skip_group_check=True)
            ts = scr.tile([P, 2 * C], BF, tag="s2c", bufs=2, name="s2cp")
            nc.vector.tensor_copy(out=ts[:], in_=ps[:])
            nc.sync.dma_start(out=t_src.ap()[sl, :], in_=ts[:])

        # ---- S3: P_dst (local, node-major) ----
        P_dst = con.tile([P, NBLK, C], F32, tag="pdst", name="pdst")
        for b in range(NBLK):
            ps = pp.tile([P, C], F32, tag="ps", name="s3ps")
            nc.tensor.matmul(out=ps[:], lhsT=(pos_locT[:, b * P:(b + 1) * P]),
                             rhs=(W_Ppos[:]), start=True, stop=True,
                             skip_group_check=True)
            nc.vector.tensor_add(out=P_dst[:, b, :], in0=ps[:],
                                 in1=b_pos_rep[:])

        # ---- S4/S5: edges ----
        xpt = con.tile([P, NBLK, C], F32, tag="xpt", name="xpt")
        for b in range(NBLK):
            bps = ppt.tile([P, 2 * C], F32, tag="tt", name="blkps")
            for t in range(TE):
                it = b * TE + t
                g = gat.tile([P, 2 * C], BF, tag="g", name="gtile")
                nc.gpsimd.indirect_dma_start(
                    out=g[:], out_offset=None, in_=t_src.ap(),
                    in_offset=bass.IndirectOffsetOnAxis(
                        ap=srcix[:, it:it + 1], axis=0))
                s = scr.tile([P, 2 * C], BF, tag="sc", bufs=3, name="stile")
                nc.scalar.activation(out=s[:, 0:C], in_=g[:, 0:C], func=AF.Exp)
                nc.vector.tensor_mul(out=s[:, C:2 * C], in0=s[:, 0:C],
                                     in1=g[:, C:2 * C])
                oh = scr.tile([P, P], BF, tag="oh", bufs=3, name="ohtile")
                nc.vector.tensor_scalar(out=oh[:], in0=iota_f[:],
                                        scalar1=dstl_f[:, it:it + 1],
                                        scalar2=None, op0=OP.is_equal)
                nc.tensor.matmul(out=bps[:], lhsT=(oh[:]), rhs=(s[:]),
                                 start=(t == 0), stop=(t == TE - 1),
                                 skip_group_check=True)
            d1 = scr.tile([P, C], F32, tag="d1", name="d1t")
            nc.scalar.activation(out=d1[:], in_=bps[:, 0:C], func=AF.Identity,
                                 bias=eps16_sb[:])
            r_ = scr.tile([P, C], F32, tag="rr", name="rrt")
            nc.vector.reciprocal(out=r_[:], in_=d1[:])
            u = scr.tile([P, C], F32, tag="uu", name="uut")
            nc.vector.tensor_mul(out=u[:], in0=bps[:, 0:C], in1=r_[:])
            t0 = scr.tile([P, C], F32, tag="t0", name="t0t")
            nc.vector.tensor_mul(out=t0[:], in0=bps[:, C:2 * C], in1=r_[:])
            nc.vector.tensor_mul(out=u[:], in0=u[:], in1=P_dst[:, b, :])
            nc.vector.tensor_add(out=xpt[:, b, :], in0=t0[:], in1=u[:])

        # ---- dense-chain helpers (channel-major) ----
        def ctile(tag, shape=(P, NSH), dt=BF):
            return chain.tile(list(shape), dt, tag=tag, name=tag + "_t")

        def transpose_nm_to_cm(src_nmc, tag):
            outs = [ctile(f"{tag}{c}") for c in range(2)]
            for b in range(NBLK):
                for c in range(2):
                    tp = ppt.tile([P, P], F32, tag="tt", name="tpt")
                    nc.tensor.transpose(out=tp[:],
                                        in_=src_nmc[:, b, c * P:(c + 1) * P],
                                        identity=id128[:])
                    nc.vector.tensor_copy(out=outs[c][:, b * P:(b + 1) * P],
                                          in_=tp[:])
            return outs

        def linear_cm(xs, wname, bname, relu=False, tag=""):
            W, B = WS[wname], BS[bname]
            outs = []
            for oc in range(2):
                ps = pp.tile([P, NSH], F32, tag="ps", name="linps")
                for ic in range(2):
                    nc.tensor.matmul(
                        out=ps[:], lhsT=(W[:, ic, oc * P:(oc + 1) * P]),
                        rhs=(xs[ic][:]), start=(ic == 0), stop=(ic == 1),
                        skip_group_check=True)
                o = ctile(f"{tag}{oc}")
                nc.scalar.activation(out=o[:], in_=ps[:],
                                     func=AF.Relu if relu else AF.Identity,
                                     bias=B[:, oc:oc + 1])
                outs.append(o)
            return outs

        def ln_cm(xs, gname, bname, tag=""):
            G, B = BS[gname], BS[bname]
            xx = [ctile(f"xx{c}") for c in range(2)]
            for c in range(2):
                nc.vector.tensor_mul(out=xx[c][:], in0=xs[c][:], in1=xs[c][:])
            mps = pp.tile([P, NSH], F32, tag="ps", name="meanps")
            eps2 = pp.tile([P, NSH], F32, tag="ps", name="ex2ps")
            for c in range(2):
                nc.tensor.matmul(out=mps[:], lhsT=(ones256[:]),
                                 rhs=(xs[c][:]), start=(c == 0),
                                 stop=(c == 1), skip_group_check=True)
            for c in range(2):
                nc.tensor.matmul(out=eps2[:], lhsT=(ones256[:]),
                                 rhs=(xx[c][:]), start=(c == 0),
                                 stop=(c == 1), skip_group_check=True)
            msq = ctile("msq", dt=F32)
            nc.scalar.activation(out=msq[:], in_=mps[:], func=AF.Square)
            var = ctile("var", dt=F32)
            nc.vector.tensor_tensor(out=var[:], in0=eps2[:], in1=msq[:],
                                    op=OP.subtract)
            nc.scalar.activation(out=var[:], in_=var[:], func=AF.Ln,
                                 bias=eps_sb[:])
            nc.scalar.activation(out=var[:], in_=var[:], func=AF.Exp,
                                 scale=-0.5)
            outs = []
            for c in range(2):
                o = ctile(f"{tag}{c}")
                nc.vector.tensor_tensor(out=o[:], in0=xs[c][:], in1=mps[:],
                                        op=OP.subtract)
                nc.vector.tensor_mul(out=o[:], in0=o[:], in1=var[:])
                nc.vector.tensor_scalar(out=o[:], in0=o[:],
                                        scalar1=G[:, c:c + 1],
                                        scalar2=B[:, c:c + 1],
                                        op0=OP.mult, op1=OP.add)
                outs.append(o)
            return outs

        def add2(xs, ys, tag):
            outs = [ctile(f"{tag}{c}") for c in range(2)]
            for c in range(2):
                nc.vector.tensor_add(out=outs[c][:], in0=xs[c][:],
                                     in1=ys[c][:])
            return outs

        # ---- chain (tags tA..tF rotate live slots) ----
        xptT = transpose_nm_to_cm(xpt, "tA")
        p1 = linear_cm(xptT, "proj_W1", "proj_b1", relu=True, tag="tB")
        p2 = linear_cm(p1, "proj_W2", "proj_b2", tag="tC")
        xlp = ln_cm(p2, "ln_l_g", "ln_l_b", tag="tA")
        l1 = linear_cm(xlp, "lm_W1", "lm_b1", relu=True, tag="tB")
        l2 = linear_cm(l1, "lm_W2", "lm_b2", tag="tC")
        xloc = add2(l2, xlp, "tD")
        hT = ln_cm(xloc, "n1_g", "n1_b", tag="tA")
        pe1p = pp.tile([PE, NSH], F32, tag="ps", name="pe1ps")
        nc.tensor.matmul(out=pe1p[:], lhsT=(pe_W1[:]), rhs=(pos_locT[:]),
                         start=True, stop=True, skip_group_check=True)
        pe1 = ctile("pe1s", (PE, NSH))
        nc.scalar.activation(out=pe1[:], in_=pe1p[:], func=AF.Relu,
                             bias=pe_b1[:])
        pe2p = pp.tile([PE, NSH], F32, tag="ps", name="pe2ps")
        nc.tensor.matmul(out=pe2p[:], lhsT=(pe_W2[:]), rhs=(pe1[:]),
                         start=True, stop=True, skip_group_check=True)
        peT = ctile("pe2s", (PE, NSH))
        nc.scalar.activation(out=peT[:], in_=pe2p[:], func=AF.Identity,
                             bias=pe_b2[:])
        hc = []
        for oc in range(2):
            ps = pp.tile([P, NSH], F32, tag="ps", name="hcps")
            W = WS["comb_Wa"]
            for ic in range(2):
                nc.tensor.matmul(out=ps[:],
                                 lhsT=(W[:, ic, oc * P:(oc + 1) * P]),
                                 rhs=(hT[ic][:]), start=(ic == 0),
                                 stop=False, skip_group_check=True)
            nc.tensor.matmul(out=ps[:],
                             lhsT=(comb_Wb[:, oc * P:(oc + 1) * P]),
                             rhs=(peT[:]), start=False, stop=True,
                             skip_group_check=True)
            o = ctile(f"tC{oc}")
            nc.scalar.activation(out=o[:], in_=ps[:], func=AF.Identity,
                                 bias=BS["comb_b"][:, oc:oc + 1])
            hc.append(o)
        qT = linear_cm(hc, "q_W", "q_b", tag="tB")
        kT = linear_cm(hc, "k_W", "k_b", tag="tE")
        vT = linear_cm(hc, "v_W", "v_b", tag="tF")

        # ---- V_aug + shard stores + AllGather ----
        vaug = big.tile([P, NBLK, 272], BF, tag="vaug", name="vaug")
        nc.vector.memset(vaug[:], 0.0)
        for h in range(H):
            hh, hp = h // 2, (h % 2) * 64
            for qt in range(NBLK):
                tp = ppt.tile([P, 64], BF, tag="tt", name="tpv")
                nc.tensor.matmul(out=tp[:],
                                 lhsT=vT[hh][hp:hp + 64, qt * P:(qt + 1) * P],
                                 rhs=id64[hp:hp + 64, :], is_transpose=True,
                                 skip_group_check=True)
                nc.vector.tensor_copy(out=vaug[:, qt, h * 68:h * 68 + 64],
                                      in_=tp[:])
        for h in range(H):
            nc.vector.memset(vaug[:, :, h * 68 + 64:h * 68 + 65], 1.0)
        nc.sync.dma_start(
            out=vaug_sh.ap().rearrange("(qt p) c -> p qt c", p=P),
            in_=vaug[:])
        for c in range(2):
            nc.sync.dma_start(out=kt_sh.ap()[c], in_=kT[c][:])
        nc.gpsimd.collective_compute(
            "AllGather", OP.bypass, replica_groups=[list(range(NC))],
            ins=[kt_sh.ap().opt()], outs=[kt_full.ap().opt()])
        nc.gpsimd.collective_compute(
            "AllGather", OP.bypass, replica_groups=[list(range(NC))],
            ins=[vaug_sh.ap().opt()], outs=[vaug_full.ap().opt()])

        # ---- SDPA ----
        oT = [ppo.tile([65, NSH], F32, tag="ot", name=f"ot{h}")
              for h in range(H)]
        for g in range(32):
            gc, lq = g // 4, g % 4
            kt_t = kv.tile([P, 2, P], BF, tag="ktt", name="ktt")
            nc.sync.dma_start(
                out=kt_t[:],
                in_=kt_full.ap()[gc, :, :, lq * P:(lq + 1) * P]
                .rearrange("hh p q -> p hh q"))
            v_t = kv.tile([P, 272], BF, tag="vtt", name="vtt")
            nc.sync.dma_start(out=v_t[:],
                              in_=vaug_full.ap()[gc, lq * P:(lq + 1) * P, :])
            for h in range(H):
                hh, hp = h // 2, (h % 2) * 64
                sps = pp.tile([P, NSH], F32, tag="ps", name="sps")
                nc.tensor.matmul(out=sps[:], lhsT=(kt_t[hp:hp + 64, hh, :]),
                                 rhs=(qT[hh][hp:hp + 64, :]),
                                 start=True, stop=True, skip_group_check=True)
                ex = scr.tile([P, NSH], BF, tag="ex", bufs=3, name="ext")
                nc.scalar.activation(out=ex[:], in_=sps[:], func=AF.Exp,
                                     scale=0.125)
                nc.tensor.matmul(out=oT[h][:],
                                 lhsT=(v_t[:, h * 68:h * 68 + 65]),
                                 rhs=(ex[:]), start=(g == 0), stop=(g == 31),
                                 skip_group_check=True)

        # ---- o epilogue: normalize per (head, node) ----
        o_nm = big.tile([P, NBLK, H, 64], F32, tag="onm", name="onm")
        dn = big.tile([P, NBLK, H, 1], F32, tag="dnm", name="dnm")
        for h in range(H):
            osb = scr.tile([65, NSH], F32, tag="osb", bufs=2, name="osbt")
            nc.vector.tensor_copy(out=osb[:], in_=oT[h][:])
            for qt in range(NBLK):
                tp = ppt.tile([P, 65], F32, tag="tt", name="tpo")
                nc.tensor.matmul(out=tp[:],
                                 lhsT=osb[:, qt * P:(qt + 1) * P],
                                 rhs=id128[0:65, 0:65], is_transpose=True,
                                 skip_group_check=True)
                nc.vector.tensor_copy(out=o_nm[:, qt, h, :], in_=tp[:, 0:64])
                nc.vector.tensor_copy(out=dn[:, qt, h, :], in_=tp[:, 64:65])
        rc = scr.tile([P, NBLK, H, 1], F32, tag="rc", name="rct")
        nc.vector.reciprocal(out=rc[:], in_=dn[:])
        for qt in range(NBLK):
            for h in range(H):
                nc.vector.tensor_scalar_mul(out=o_nm[:, qt, h, :],
                                            in0=o_nm[:, qt, h, :],
                                            scalar1=rc[:, qt, h, :])
        oTn = [ctile(f"tC{c}") for c in range(2)]
        for c in range(2):
            for qt in range(NBLK):
                tp = ppt.tile([P, P], F32, tag="tt", name="tpb")
                nc.tensor.transpose(out=tp[:],
                                    in_=o_nm[:, qt, c * 2:c * 2 + 2, :],
                                    identity=id128[:])
                nc.vector.tensor_copy(out=oTn[c][:, qt * P:(qt + 1) * P],
                                      in_=tp[:])
        xo = linear_cm(oTn, "o_W", "o_b", tag="tE")
        x2 = add2(xo, xloc, "tA")
        xg1 = ln_cm(x2, "n2_g", "n2_b", tag="tE")
        g1 = linear_cm(xg1, "gm_W1", "gm_b1", relu=True, tag="tF")
        g2 = linear_cm(g1, "gm_W2", "gm_b2", tag="tB")
        xgT = add2(g2, xg1, "tD")

        out_sb = big.tile([P, NBLK, C], F32, tag="osb2", name="outsb")
        for c in range(2):
            for qt in range(NBLK):
                tp = ppt.tile([P, P], BF, tag="tt", name="tpf")
                nc.tensor.transpose(out=tp[:],
                                    in_=xgT[c][:, qt * P:(qt + 1) * P],
                                    identity=id128b[:])
                nc.vector.tensor_copy(out=out_sb[:, qt, c * P:(c + 1) * P],
                                      in_=tp[:])
        nc.sync.dma_start(
            out=xg_out.ap().rearrange("(qt p) c -> p qt c", p=P),
            in_=out_sb[:])
    return nc


def kernel(**inputs):
    shared, per_core, TE, NTE = _pack_host(inputs)
    nc = build(TE, NTE)
    _split_waits(nc)
    in_maps = []
    for ci in range(NC):
        m = dict(shared)
        m.update(per_core[ci])
        in_maps.append(m)
    import os
    trace = os.environ.get("KTRACE", "0") == "1"
    res = bass_utils.run_bass_kernel_spmd(nc, in_maps,
                                          core_ids=list(range(NC)),
                                          trace=trace)
    if trace:
        print("HW exec time:", res.exec_time_ns, "ns")
        if res.instructions_and_trace:
            print("trace:", res.instructions_and_trace[1])
    xg = np.concatenate([res.results[ci]["xg_out"] for ci in range(NC)], 0)
    return (xg, np.asarray(inputs["pos"], np.float32))
